# revision 1
# baseline (speedup 1.0000x reference)
"""Trainium2 Bass kernel for additive-attention GNN message passing.

reference semantics (NV=4096 nodes, NE=32768 edges, DV=256, DATTN=128):
    Zi = nodes[s] @ Z.T ; Zj = nodes[r] @ Z.T
    pre = leaky_relu(concat([Zi, Zj], 1) @ w, 0.01)
    segment softmax of pre over receiver groups r
    out[r_e, e] = softmax_val_e   (dense (NV, NE), zeros elsewhere)

Key algebra: concat([Zi,Zj]) @ w == nodes[s] @ (Z.T @ w1) + nodes[r] @ (Z.T @ w2)
so per-edge scores collapse to alpha[s_e] + beta[r_e] with alpha = nodes @ a,
beta = nodes @ b, a = Z.T @ w[:128], b = Z.T @ w[128:].  pre is bounded
(|alpha+beta| <~ 6) so the softmax needs no max subtraction.

Sharding: edges across 8 cores (4096 each); nodes/Z/w replicated; AllReduce
of the per-node exp-sum denominator.  Each core writes its dense
(4096 edges, 4096 nodes) block = transpose of its slice of the output.
"""

import numpy as np

import concourse.bacc as bacc
import concourse.bass as bass
import concourse.mybir as mybir
import concourse.tile as tile
from concourse.bass_utils import run_bass_kernel_spmd

P = 128
PADW = 64  # table row padding: 64 f32 = 256B (dma_gather elem granularity)
NEG_SLOPE = 0.01

_cached = {}


def build(nv, dv, dattn, nel, n_cores, debug=False, stage=99, timing=False, krep=1, skip_zero=False):
    """Build the SPMD Bacc graph for one core (replicated across n_cores).

    nel = local edges per core. Layout conventions:
      - edge e_local <-> (p = e%128, c = e//128); per-edge SBUF arrays are (128, C).
      - node tables in DRAM use "u-order": row u = pi(v) = (v%128)*nt + v//128,
        so (128, nt) chunk-major SBUF node arrays DMA contiguously to tables.
      - gather indices (host-precomputed) are pi-mapped.
    """
    C = nel // P       # edge chunks
    nt = nv // P       # node chunks
    NKC = nv // 512    # 512-wide column chunks for matmul
    f32 = mybir.dt.float32
    i16 = mybir.dt.int16
    alu = mybir.AluOpType
    act = mybir.ActivationFunctionType

    nc = bacc.Bacc("TRN2", target_bir_lowering=False, debug=False,
                   num_devices=n_cores)

    nodes_d = nc.dram_tensor("nodes", [nv, dv], f32, kind="ExternalInput")
    z_d = nc.dram_tensor("Zm", [dattn, dv], f32, kind="ExternalInput")
    w_d = nc.dram_tensor("w_col", [dattn, 2], f32, kind="ExternalInput")
    iota_d = nc.dram_tensor("iota_row", [1, nv], f32, kind="ExternalInput")
    idxs_d = nc.dram_tensor("idx_s", [P, nel // 16], i16, kind="ExternalInput")
    idxr_d = nc.dram_tensor("idx_r", [P, nel // 16], i16, kind="ExternalInput")
    rpif_d = nc.dram_tensor("r_pi_f", [P, C], f32, kind="ExternalInput")
    rnatf_d = nc.dram_tensor("r_nat_f", [P, C], f32, kind="ExternalInput")
    rpi16_d = nc.dram_tensor("rpi16", [P, C], i16, kind="ExternalInput")
    iota16_d = nc.dram_tensor("iota16", [1, nv], i16, kind="ExternalInput")
    rmodf_d = nc.dram_tensor("rmodf", [P, C], f32, kind="ExternalInput")
    GE_in = min(32768 // (nv // 64), nel)
    scidx_d = nc.dram_tensor("scidx", [nel // GE_in, P, GE_in // 16],
                             i16, kind="ExternalInput")
    if timing:
        # Timing variant: identical device work, but the big output lands in
        # internal DRAM so per-call host/result overhead is tiny.
        out_d = nc.dram_tensor("out_scr", [nel, nv], f32, kind="Internal")
        tout_d = nc.dram_tensor("tout", [P, 4], f32, kind="ExternalOutput")
    else:
        out_d = nc.dram_tensor("out", [nel, nv], f32, kind="ExternalOutput")
    if debug:
        dbg = {
            "d_alpha": nc.dram_tensor("d_alpha", [P, nv // P], f32, kind="ExternalOutput"),
            "d_beta": nc.dram_tensor("d_beta", [P, nv // P], f32, kind="ExternalOutput"),
            "d_als": nc.dram_tensor("d_als", [P, nel // P], f32, kind="ExternalOutput"),
            "d_bet": nc.dram_tensor("d_bet", [P, nel // P], f32, kind="ExternalOutput"),
            "d_ex": nc.dram_tensor("d_ex", [P, nel // P], f32, kind="ExternalOutput"),
            "d_dn": nc.dram_tensor("d_dn", [P, nv // P], f32, kind="ExternalOutput"),
            "d_invd": nc.dram_tensor("d_invd", [P, nv // P], f32, kind="ExternalOutput"),
            "d_vals": nc.dram_tensor("d_vals", [P, nel // P], f32, kind="ExternalOutput"),
        }

    with tile.TileContext(nc) as tc:
        with tc.tile_pool(name="const", bufs=1) as cp, \
             tc.tile_pool(name="dram", bufs=1, space="DRAM") as dp:

            for _kr in range(krep):
                # ---- small loads -------------------------------------------------
                z_sb = cp.tile([dattn, dv], f32)
                nc.sync.dma_start(z_sb[:], z_d.ap())
                w_sb = cp.tile([dattn, 2], f32)
                nc.sync.dma_start(w_sb[:], w_d.ap())
                iota_row = cp.tile([1, nv], f32)
                nc.sync.dma_start(iota_row[:], iota_d.ap())
                rpif = cp.tile([P, C], f32)
                nc.sync.dma_start(rpif[:], rpif_d.ap())
                if not skip_zero:
                    rnatf = cp.tile([P, C], f32)
                    nc.sync.dma_start(rnatf[:], rnatf_d.ap())
                idxs_sb = cp.tile([P, nel // 16], i16)
                nc.sync.dma_start(idxs_sb[:], idxs_d.ap())
                idxr_sb = cp.tile([P, nel // 16], i16)
                nc.sync.dma_start(idxr_sb[:], idxr_d.ap())
                rpi16 = cp.tile([P, C], i16)
                nc.sync.dma_start(rpi16[:], rpi16_d.ap())
                rmodf = cp.tile([P, C], f32)
                nc.sync.dma_start(rmodf[:], rmodf_d.ap())
                ngrp_l = scidx_d.shape[0]
                scidx_sb = []
                for g in range(ngrp_l):
                    sct = cp.tile([P, scidx_d.shape[2]], i16, tag=f"scidx{g}")
                    nc.sync.dma_start(sct[:], scidx_d.ap()[g])
                    scidx_sb.append(sct)
                iota16_row = cp.tile([1, nv], i16)
                nc.sync.dma_start(iota16_row[:], iota16_d.ap())
                iota16_bc = cp.tile([P, nv], i16)
                nc.gpsimd.partition_broadcast(iota16_bc[:], iota16_row[:])
                ones_row = cp.tile([1, P], f32)
                nc.vector.memset(ones_row[:], 1.0)

                # ---- PE: a = Z.T @ w1, b = Z.T @ w2 (as rows); broadcasts -------
                a_row = cp.tile([1, dv], f32)
                b_row = cp.tile([1, dv], f32)
                if skip_zero:
                    iota64_bc = cp.tile([P, 64], f32)
                    iota_bc = None
                else:
                    iota_bc = cp.tile([P, nv], f32)
                a_bc = cp.tile([P, dv], f32)
                b_bc = cp.tile([P, dv], f32)
                with tc.tile_pool(name=f"psA{_kr}", bufs=2, space="PSUM") as pA:
                    abp = pA.tile([1, dv], f32, tag="abp")
                    nc.tensor.matmul(abp[:], lhsT=w_sb[:, 0:1], rhs=z_sb[:],
                                     start=True, stop=True)
                    nc.vector.tensor_copy(a_row[:], abp[:])
                    abp2 = pA.tile([1, dv], f32, tag="abp")
                    nc.tensor.matmul(abp2[:], lhsT=w_sb[:, 1:2], rhs=z_sb[:],
                                     start=True, stop=True)
                    nc.vector.tensor_copy(b_row[:], abp2[:])
                    # broadcast a/b rows across partitions via ones-matmul
                    ap_ = pA.tile([P, dv], f32, tag="bcast")
                    nc.tensor.matmul(ap_[:], lhsT=ones_row[:], rhs=a_row[:],
                                     start=True, stop=True)
                    nc.vector.tensor_copy(a_bc[:], ap_[:])
                    bp_ = pA.tile([P, dv], f32, tag="bcast")
                    nc.tensor.matmul(bp_[:], lhsT=ones_row[:], rhs=b_row[:],
                                     start=True, stop=True)
                    nc.vector.tensor_copy(b_bc[:], bp_[:])
                    # iota broadcast: skip-zero only needs columns 0..63
                    if skip_zero:
                        i64p = pA.tile([P, 64], f32, tag="iob")
                        nc.tensor.matmul(i64p[:], lhsT=ones_row[:],
                                         rhs=iota_row[:, 0:64],
                                         start=True, stop=True)
                        nc.vector.tensor_copy(iota64_bc[:], i64p[:])
                    else:
                        for k in range(NKC):
                            ip_ = pA.tile([P, 512], f32, tag="iob")
                            nc.tensor.matmul(ip_[:], lhsT=ones_row[:],
                                             rhs=iota_row[:, k * 512:(k + 1) * 512],
                                             start=True, stop=True)
                            nc.vector.tensor_copy(
                                iota_bc[:, k * 512:(k + 1) * 512], ip_[:])

                if stage >= 2:
                    # ---- nodes load + alpha/beta ------------------------------------
                    alpha = cp.tile([P, nt], f32)
                    beta = cp.tile([P, nt], f32)
                    with tc.tile_pool(name=f"nod{_kr}", bufs=4) as npool:
                        for t in range(nt):
                            ng = npool.tile([P, dv], f32, tag="nodes")
                            nc.sync.dma_start(ng[:], nodes_d.ap()[t * P:(t + 1) * P, :])
                            scr = npool.tile([P, dv], f32, tag="scr")
                            nc.vector.tensor_tensor(out=scr[:], in0=ng[:],
                                                    in1=a_bc[:], op=alu.mult)
                            nc.vector.reduce_sum(alpha[:, t:t + 1], scr[:],
                                                 axis=mybir.AxisListType.X)
                            scr2 = npool.tile([P, dv], f32, tag="scr")
                            nc.vector.tensor_tensor(out=scr2[:], in0=ng[:],
                                                    in1=b_bc[:], op=alu.mult)
                            nc.vector.reduce_sum(beta[:, t:t + 1], scr2[:],
                                                 axis=mybir.AxisListType.X)

                if stage >= 3:
                    # ---- alpha/beta padded table + gathers --------------------------
                    pad = cp.tile([P, nt * PADW], f32)
                    nc.vector.memset(pad[:], 0.0)
                    nc.vector.tensor_copy(pad[:, 0:nt * PADW:PADW], alpha[:])
                    nc.vector.tensor_copy(pad[:, 1:nt * PADW:PADW], beta[:])
                    ab_tab = dp.tile([nv, PADW], f32)
                    nc.sync.dma_start(ab_tab[:].rearrange("(p t) j -> p (t j)", p=P),
                                      pad[:])
                    als = cp.tile([P, C, PADW], f32)
                    nc.gpsimd.dma_gather(als[:], ab_tab[:], idxs_sb[:], nel, nel, PADW, single_packet=False)
                    bets = cp.tile([P, C, PADW], f32)
                    nc.gpsimd.dma_gather(bets[:], ab_tab[:], idxr_sb[:], nel, nel, PADW, single_packet=False)

                    # ---- per-edge: ex = exp(leaky_relu(alpha_s + beta_r)) -----------
                    pre = cp.tile([P, C], f32)
                    nc.vector.tensor_tensor(out=pre[:], in0=als[:, :, 0],
                                            in1=bets[:, :, 1], op=alu.add)
                    pre_s = cp.tile([P, C], f32)
                    nc.vector.tensor_scalar(out=pre_s[:], in0=pre[:],
                                            scalar1=NEG_SLOPE, scalar2=None,
                                            op0=alu.mult)
                    pre2 = cp.tile([P, C], f32)
                    nc.vector.tensor_tensor(out=pre2[:], in0=pre[:], in1=pre_s[:],
                                            op=alu.max)
                    ex = cp.tile([P, C], f32)
                    nc.scalar.activation(ex[:], pre2[:], act.Exp)

                if stage >= 4:
                    # ---- pass 1: local denominator via one-hot mask matmul ----------
                    # bf16 masks: {0,1} exact in bf16, and bf16 matmul streams at
                    # full rate (fp32 is 4x slower on PE); int16 iota compare gets
                    # the DVE 4x mode.  ex goes in as the bf16 stationary operand.
                    bf16 = mybir.dt.bfloat16
                    ex_bf = cp.tile([P, C], bf16)
                    nc.vector.tensor_copy(ex_bf[:], ex[:])
                    with tc.tile_pool(name=f"psD{_kr}", bufs=1, space="PSUM") as pD, \
                         tc.tile_pool(name=f"mask{_kr}", bufs=3) as mp:
                        dn = pD.tile([1, nv], f32)
                        for t in range(C):
                            m = mp.tile([P, nv], bf16, tag="mask")
                            nc.vector.tensor_scalar(
                                out=m[:], in0=iota16_bc[:],
                                scalar1=rpif[:, t:t + 1],
                                scalar2=None, op0=alu.is_equal)
                            for k in range(NKC):
                                nc.tensor.matmul(dn[0:1, k * 512:(k + 1) * 512],
                                                 lhsT=ex_bf[:, t:t + 1],
                                                 rhs=m[:, k * 512:(k + 1) * 512],
                                                 start=(t == 0), stop=(t == C - 1))
                        dn_sb = cp.tile([1, nv], f32)
                        nc.vector.tensor_copy(dn_sb[:], dn[:])

                if stage >= 5:
                    # ---- AllReduce denominator across cores -------------------------
                    cc_in = dp.tile([1, nv], f32)
                    nc.gpsimd.dma_start(cc_in[:], dn_sb[:])
                    cc_out = dp.tile([P, nt], f32)
                    nc.gpsimd.collective_compute(
                        "AllReduce", alu.add,
                        replica_groups=[list(range(n_cores))],
                        ins=[cc_in[:].opt()], outs=[cc_out[:].opt()])

                    # ---- invd table + gather; vals ----------------------------------
                    dng = cp.tile([P, nt], f32)
                    nc.sync.dma_start(dng[:], cc_out[:])
                    dnc = cp.tile([P, nt], f32)
                    nc.vector.tensor_scalar(out=dnc[:], in0=dng[:], scalar1=1e-30,
                                            scalar2=None, op0=alu.max)
                    invd = cp.tile([P, nt], f32)
                    nc.vector.reciprocal(invd[:], dnc[:])
                    pad2 = cp.tile([P, nt * PADW], f32)
                    nc.vector.memset(pad2[:], 0.0)
                    nc.vector.tensor_copy(pad2[:, 0:nt * PADW:PADW], invd[:])
                    ivd_tab = dp.tile([nv, PADW], f32)
                    nc.sync.dma_start(ivd_tab[:].rearrange("(p t) j -> p (t j)", p=P),
                                      pad2[:])
                    ivg = cp.tile([P, C, PADW], f32)
                    nc.gpsimd.dma_gather(ivg[:], ivd_tab[:], idxr_sb[:], nel, nel, PADW, single_packet=False)
                    vals = cp.tile([P, C], f32)
                    nc.vector.tensor_tensor(out=vals[:], in0=ex[:], in1=ivg[:, :, 0],
                                            op=alu.mult)

                    if debug:
                        nc.sync.dma_start(dbg["d_alpha"].ap(), alpha[:])
                        nc.sync.dma_start(dbg["d_beta"].ap(), beta[:])
                        nc.sync.dma_start(dbg["d_als"].ap(), als[:, :, 0])
                        nc.sync.dma_start(dbg["d_bet"].ap(), bets[:, :, 1])
                        nc.sync.dma_start(dbg["d_ex"].ap(), ex[:])
                        nc.sync.dma_start(dbg["d_dn"].ap(), dng[:])
                        nc.sync.dma_start(dbg["d_invd"].ap(), invd[:])
                        nc.sync.dma_start(dbg["d_vals"].ap(), vals[:])

                if stage >= 6:
                    # ---- pass 2: emit output ---------------------------------------
                    if skip_zero:
                        # Runtime pre-zeros ExternalOutput buffers (documented
                        # contract both in run_bass_kernel_spmd and the PJRT
                        # donation path), so only scatter the 1 nonzero per
                        # edge.  dma_scatter_add writes 256B blocks; block id
                        # e*(nv//64) + r//64 is unique per edge, so += on a
                        # zero dest is an exact write.  8 groups keep block
                        # ids within int16.
                        NB = nv // 64               # 256B blocks per row
                        GE = min(32768 // NB, nel)  # edges per group (int16 cap)
                        ngrp = nel // GE
                        srcblk = cp.tile([P, C, 64], f32)
                        for c in range(C):
                            nc.vector.tensor_scalar(
                                out=srcblk[:, c, :], in0=iota64_bc[:],
                                scalar1=rmodf[:, c:c + 1],
                                scalar2=vals[:, c:c + 1],
                                op0=alu.is_equal, op1=alu.mult)
                        out_blocks = out_d.ap().rearrange(
                            "e (b j) -> (e b) j", j=64)
                        CPG = GE // P               # edge chunks per group
                        for g in range(ngrp):
                            nc.gpsimd.dma_scatter_add(
                                out_blocks[g * GE * NB:(g + 1) * GE * NB, :],
                                srcblk[:, g * CPG:(g + 1) * CPG, :],
                                scidx_sb[g], GE, GE, 64,
                                single_packet=False)
                    else:
                        with tc.tile_pool(name=f"ob{_kr}", bufs=3) as op_:
                            for t in range(C):
                                ob = op_.tile([P, nv], f32, tag="ob")
                                nc.vector.tensor_scalar(
                                    out=ob[:], in0=iota_bc[:],
                                    scalar1=rnatf[:, t:t + 1],
                                    scalar2=vals[:, t:t + 1],
                                    op0=alu.is_equal, op1=alu.mult)
                                nc.sync.dma_start(
                                    out_d.ap()[t * P:(t + 1) * P, :], ob[:])

            if stage < 6 and not skip_zero:
                nc.sync.dma_start(out_d.ap()[0:P, :], iota_bc[:])

            if timing:
                tsrc = iota64_bc if skip_zero else iota_bc
                nc.sync.dma_start(tout_d.ap(), tsrc[:, 0:4])


    nc.compile()
    return nc


def build_null(nv, dv, dattn, nel, n_cores):
    """Do-nothing timing baseline: same inputs, tiny output, ~zero work."""
    f32 = mybir.dt.float32
    i16 = mybir.dt.int16
    nc = bacc.Bacc("TRN2", target_bir_lowering=False, debug=False,
                   num_devices=n_cores)
    C = nel // P
    nc.dram_tensor("nodes", [nv, dv], f32, kind="ExternalInput")
    nc.dram_tensor("Zm", [dattn, dv], f32, kind="ExternalInput")
    nc.dram_tensor("w_col", [dattn, 2], f32, kind="ExternalInput")
    nc.dram_tensor("iota_row", [1, nv], f32, kind="ExternalInput")
    nc.dram_tensor("idx_s", [P, nel // 16], i16, kind="ExternalInput")
    nc.dram_tensor("idx_r", [P, nel // 16], i16, kind="ExternalInput")
    nc.dram_tensor("r_pi_f", [P, C], f32, kind="ExternalInput")
    nc.dram_tensor("r_nat_f", [P, C], f32, kind="ExternalInput")
    tout_d = nc.dram_tensor("tout", [P, 4], f32, kind="ExternalOutput")
    with tile.TileContext(nc) as tc:
        with tc.tile_pool(name="sb", bufs=1) as sp:
            z0 = sp.tile([P, 4], f32)
            nc.vector.memset(z0[:], 0.0)
            nc.sync.dma_start(tout_d.ap(), z0[:])
    nc.compile()
    return nc


def prep_inputs(nodes, Z, w, edge_index, n_cores):
    """Host-side shard prep. Returns in_maps (list of dicts, one per core)."""
    nv, dv = nodes.shape
    dattn = Z.shape[0]
    ne = edge_index.shape[1]
    nel = ne // n_cores
    nt = nv // P

    nodes = np.ascontiguousarray(nodes, dtype=np.float32)
    Z = np.ascontiguousarray(Z, dtype=np.float32)
    w_col = np.ascontiguousarray(
        w.astype(np.float32).reshape(2, dattn).T)          # (dattn, 2)
    iota_row = np.arange(nv, dtype=np.float32).reshape(1, nv)

    in_maps = []
    for i in range(n_cores):
        s = edge_index[0, i * nel:(i + 1) * nel].astype(np.int64)
        r = edge_index[1, i * nel:(i + 1) * nel].astype(np.int64)
        s_pi = (s % P) * nt + s // P
        r_pi = (r % P) * nt + r // P
        # dma_gather places tab[idx[j]] at out[p, c] with
        #   j = (p%16)*npr + c*8 + p//16,  npr = nel//16  (measured swizzle).
        # We want out[p, c] = edge e = c*128 + p, so idx[j] = pi(ind[e(j)]).
        npr = nel // 16
        jj = np.arange(nel)
        e_of_j = ((jj % npr) // 8) * P + (jj % 8) * 16 + jj // npr
        idx_s = np.tile(s_pi[e_of_j].astype(np.int16).reshape(16, npr), (8, 1))
        idx_r = np.tile(r_pi[e_of_j].astype(np.int16).reshape(16, npr), (8, 1))
        # per-edge (128, C) chunk-major layouts: e <-> (p=e%128, c=e//128)
        r_pi_f = r_pi.astype(np.float32).reshape(-1, P).T.copy()   # (128, C)
        r_nat_f = r.astype(np.float32).reshape(-1, P).T.copy()     # (128, C)
        rpi16 = r_pi.astype(np.int16).reshape(-1, P).T.copy()      # (128, C)
        rmodf = (r % 64).astype(np.float32).reshape(-1, P).T.copy()  # (128, C)
        # scatter-add indices: 256B block id, grouped to fit int16, swizzled
        # like dma_gather: idx j pairs with src[p, ch], j = (p%16)*npr + ch*8
        # + p//16 (npr = GE//16)
        NB = nv // 64
        GE = min(32768 // NB, nel)
        ngrp = nel // GE
        npr_g = GE // 16
        jj_g = np.arange(GE)
        p_j = (jj_g % 8) * 16 + jj_g // npr_g
        ch_j = (jj_g % npr_g) // 8
        el_j = ch_j * P + p_j            # group-local edge index
        scidx = np.zeros((ngrp, P, npr_g), np.int16)
        for g in range(ngrp):
            e_glob = g * GE + el_j
            b_loc = el_j * NB + (r[e_glob] // 64)
            scidx[g] = np.tile(b_loc.astype(np.int16).reshape(16, npr_g),
                               (8, 1))
        in_maps.append({
            "nodes": nodes,
            "Zm": Z,
            "w_col": w_col,
            "iota_row": iota_row,
            "idx_s": idx_s,
            "idx_r": idx_r,
            "r_pi_f": r_pi_f,
            "r_nat_f": r_nat_f,
            "rpi16": rpi16,
            "iota16": np.arange(nv, dtype=np.int16).reshape(1, nv),
            "rmodf": rmodf,
            "scidx": scidx,
        })
    return in_maps


def run(nodes, Z, w, edge_index, n_cores=8, trace=False, **kw):
    nv, dv = nodes.shape
    dattn = Z.shape[0]
    ne = edge_index.shape[1]
    nel = ne // n_cores
    key = (nv, dv, dattn, nel, n_cores)
    if key not in _cached:
        # skip_zero relies on the runtime's documented pre-zeroing of
        # ExternalOutput buffers (run_bass_kernel_spmd native path pre-zeros;
        # the PJRT path donates fresh zero buffers per call) — verified on HW
        # with repeated invocations.
        _cached[key] = build(*key, skip_zero=True)
    nc = _cached[key]
    in_maps = prep_inputs(nodes, Z, w, edge_index, n_cores)
    res = run_bass_kernel_spmd(nc, in_maps, core_ids=list(range(n_cores)),
                               trace=trace, **kw)
    blocks = [res.results[i]["out"] for i in range(n_cores)]   # (nel, nv) each
    full = np.concatenate(blocks, axis=0)                      # (ne, nv)
    return full.T, res                                         # (nv, ne)


def kernel(nodes, Z, w, edge_index):
    out, _ = run(np.asarray(nodes), np.asarray(Z), np.asarray(w),
                 np.asarray(edge_index), n_cores=8)
    return np.ascontiguousarray(out)

